# revision 13
# baseline (speedup 1.0000x reference)
"""Multi-Head Latent Attention (MLA) TRN2 Bass kernel, 8-core parallel.

Sharding: batch x heads. Cores 0-3 own batch 0, cores 4-7 batch 1; within a
batch group each core owns 4 heads (tensor-parallel on q/kv_up/o_proj).
Each core computes the latent projection for its batch (4x replicated),
q/kv projections for its heads, attention, and a partial o_proj; the host
sums the 4 partials per batch and stacks the batches.

Dataflow is fully "transposed" so no on-device transposes are needed:
  xT [D, S] (host-side transpose, per batch) ->
  latentT = Wdown^T xT, qT = Wq^T xT, kvT = Wup^T latentT   (all [*, S])
  kv_nat = latentT^T Wup                                    ([S, *])
  scoresT[keys, q] = kvT^T(slice) qT;  expT = exp(scoresT * scale)
  outT[Dh, q]  = kv_nat^T(slice) expT  (accumulate over key tiles)
  denom[*, q]  = ones^T expT           (accumulate over key tiles)
  outT_norm    = outT * (1/denom);  final = outT_norm^T Wo   ([S, D])
Softmax max-subtraction is skipped: scores are ~N(0, 0.037), |s| < ~1.5.

The 4 heads run in 2 groups of 2 to fit SBUF; qT is staged through internal
DRAM; o_proj runs per group (group 0 writes, group 1 DMA-accumulates into
the pre-zeroed output) and is interleaved per q-chunk to spread the output
DMA and keep the PE dense (HAM-warm).

All matmuls run in float32r (fp32 with 11-bit mantissa, full PE speed).
"""
import sys

sys.path.insert(0, "/opt/trn_rl_repo")

import numpy as np  # noqa: E402

B = 2
S = 2048
D = 2048
H = 16
DH = 128
DL = 512
P = 128
N_CORES = 8
H_LOC = 4                     # heads per core
HW = H_LOC * DH               # 512
N_G = 2                       # head groups per core
G_H = H_LOC // N_G            # 2 heads per group
GW = G_H * DH                 # 256
SCALE = float(1.0 / np.sqrt(DH))

D_T = D // P                  # 16
L_T = DL // P                 # 4
S_SL = 256                    # projection s-slice width
N_SL = S // S_SL              # 8
QT_W = 512
N_QP = S // (2 * QT_W)        # 2 qt-pairs
KT = S // P                   # 16
ST = S // P                   # 16
NT_W = 512
N_NT = D // NT_W              # 4


def _round_fp32r(x: np.ndarray) -> np.ndarray:
    """Round fp32 to fp32r (11-bit mantissa, RNE), matching device rounding."""
    u = np.ascontiguousarray(x, dtype=np.float32).view(np.uint32)
    low = u & np.uint32(0x00000FFF)
    lsb = (u >> np.uint32(12)) & np.uint32(1)
    round_up = (low > np.uint32(0x800)) | ((low == np.uint32(0x800)) & (lsb == 1))
    out = (u & np.uint32(0xFFFFF000)) + (round_up.astype(np.uint32) << np.uint32(12))
    return out.view(np.float32)


def _build_nc():
    import concourse.tile as tile
    import concourse.mybir as mybir
    from concourse import bacc

    f32 = mybir.dt.float32
    f32r = mybir.dt.float32r
    EXP = mybir.ActivationFunctionType.Exp

    nc = bacc.Bacc("TRN2", target_bir_lowering=False, debug=False)

    xT = nc.dram_tensor("xT", [D, S], f32r, kind="ExternalInput").ap()
    wq = nc.dram_tensor("wq", [D, HW], f32r, kind="ExternalInput").ap()
    wdown = nc.dram_tensor("wdown", [D, DL], f32r, kind="ExternalInput").ap()
    wup = nc.dram_tensor("wup", [DL, HW], f32r, kind="ExternalInput").ap()
    wo = nc.dram_tensor("wo", [HW, D], f32r, kind="ExternalInput").ap()
    ones_d = nc.dram_tensor("ones", [P, P], f32r, kind="ExternalInput").ap()
    outs = [nc.dram_tensor(f"out{g}", [S, D], f32, kind="ExternalOutput").ap()
            for g in range(N_G)]

    with tile.TileContext(nc) as tc:
        with tc.tile_pool(name="w", bufs=1) as wp, \
             tc.tile_pool(name="xs", bufs=1) as xsp, \
             tc.tile_pool(name="big", bufs=1) as bigp, \
             tc.tile_pool(name="sm", bufs=1) as smp, \
             tc.tile_pool(name="dr", bufs=1, space="DRAM") as drp, \
             tc.tile_pool(name="ps", bufs=1, space="PSUM") as psp:

            qTd = drp.tile([HW, S], f32r, tag="qTd", name="qTd")

            # ---- weights + first-slice xs, in need order ----
            wdown_t = []
            xs0 = []
            for dt_i in range(D_T):
                t = wp.tile([P, DL], f32r, tag=f"wd_{dt_i}", name=f"wd_{dt_i}")
                nc.sync.dma_start(t[:], wdown[dt_i * P:(dt_i + 1) * P, :])
                wdown_t.append(t)
                t = xsp.tile([P, S_SL], f32r, tag=f"xs_{dt_i}", bufs=2,
                             name=f"xs_0_{dt_i}")
                nc.sync.dma_start(t[:], xT[dt_i * P:(dt_i + 1) * P, 0:S_SL])
                xs0.append(t)
            wq_t = []
            for dt_i in range(D_T):
                t = wp.tile([P, HW], f32r, tag=f"wq_{dt_i}", name=f"wq_{dt_i}")
                nc.sync.dma_start(t[:], wq[dt_i * P:(dt_i + 1) * P, :])
                wq_t.append(t)
            ones_t = wp.tile([P, P], f32r, tag="ones", name="ones")
            nc.sync.dma_start(ones_t[:], ones_d[:, :])

            latT = [bigp.tile([P, S], f32r, tag=f"latT_{m}", name=f"latT_{m}")
                    for m in range(L_T)]

            # ---- Phase A0: latent + q projections, streamed over s-slices
            for j in range(N_SL):
                if j == 0:
                    xs = xs0
                else:
                    xs = []
                    for dt_i in range(D_T):
                        t = xsp.tile([P, S_SL], f32r, tag=f"xs_{dt_i}", bufs=2,
                                     name=f"xs_{j}_{dt_i}")
                        nc.sync.dma_start(t[:], xT[dt_i * P:(dt_i + 1) * P,
                                                   j * S_SL:(j + 1) * S_SL])
                        xs.append(t)
                jsl = slice(j * S_SL, (j + 1) * S_SL)
                for m in range(L_T):
                    ps = psp.tile([P, 2 * QT_W], f32, tag="sc", bufs=2,
                                  name=f"psA_{j}_l{m}")
                    for dt_i in range(D_T):
                        nc.tensor.matmul(ps[:, :S_SL],
                                         wdown_t[dt_i][:, m * P:(m + 1) * P],
                                         xs[dt_i][:],
                                         start=(dt_i == 0),
                                         stop=(dt_i == D_T - 1))
                    nc.vector.tensor_copy(latT[m][:, jsl], ps[:, :S_SL])
                for m in range(H_LOC):
                    ps = psp.tile([P, 2 * QT_W], f32, tag="sc", bufs=2,
                                  name=f"psA_{j}_q{m}")
                    for dt_i in range(D_T):
                        nc.tensor.matmul(ps[:, :S_SL],
                                         wq_t[dt_i][:, m * P:(m + 1) * P],
                                         xs[dt_i][:],
                                         start=(dt_i == 0),
                                         stop=(dt_i == D_T - 1))
                    qs_t = smp.tile([P, S_SL], f32r, tag="qstage", bufs=3,
                                    name=f"qstage_{j}_{m}")
                    nc.vector.tensor_copy(qs_t[:], ps[:, :S_SL])
                    nc.gpsimd.dma_start(qTd[m * P:(m + 1) * P, jsl], qs_t[:])

            # ---- per head-group: kv projections, attention, o_proj ----
            # o_proj chains are queued and drained one-per-2-kt inside later
            # attention loops (and the next group's kv loop) so their PE/DVE/
            # ACT/DMA work fills dependency-stall gaps instead of serializing.
            pending = []

            for g in range(N_G):
                gsl = slice(g * GW, (g + 1) * GW)
                wup_t = []
                for lt in range(L_T):
                    t = wp.tile([P, GW], f32r, tag=f"wu_{lt}", name=f"wu_{g}_{lt}")
                    nc.sync.dma_start(t[:], wup[lt * P:(lt + 1) * P, gsl])
                    wup_t.append(t)

                kvT = [bigp.tile([P, S], f32r, tag=f"kvT_{m}", name=f"kvT_{g}_{m}")
                       for m in range(G_H)]
                # kv_nat shares the wd_* weight slots (free after Phase A0)
                kvn = [wp.tile([P, GW], f32r, tag=f"wd_{st}", name=f"kvn_{g}_{st}")
                       for st in range(ST)]
                outT = [bigp.tile([P, S], f32r, tag=f"outT_{m}",
                                  name=f"outT_{g}_{m}")
                        for m in range(G_H)]

                # Phase A1(g): kv projections from latT, drain pending o_proj
                def _kvt_chain(g, j, m, kvT=kvT, wup_t=wup_t):
                    jsl = slice(j * S_SL, (j + 1) * S_SL)
                    ps = psp.tile([P, 2 * QT_W], f32, tag="sc", bufs=2,
                                  name=f"psK_{g}_{j}_{m}")
                    for lt in range(L_T):
                        nc.tensor.matmul(ps[:, :S_SL],
                                         wup_t[lt][:, m * P:(m + 1) * P],
                                         latT[lt][:, jsl],
                                         start=(lt == 0), stop=(lt == L_T - 1))
                    nc.vector.tensor_copy(kvT[m][:, jsl], ps[:, :S_SL])

                def _kvn_chain(g, st, kvn=kvn, wup_t=wup_t):
                    ps = psp.tile([P, 2 * QT_W], f32, tag="sc", bufs=2,
                                  name=f"psN_{g}_{st}")
                    for lt in range(L_T):
                        nc.tensor.matmul(ps[:, :GW],
                                         latT[lt][:, st * P:(st + 1) * P],
                                         wup_t[lt][:],
                                         start=(lt == 0), stop=(lt == L_T - 1))
                    nc.vector.tensor_copy(kvn[st][:], ps[:, :GW])

                a1 = []
                for j in range(N_SL):
                    for m in range(G_H):
                        a1.append((lambda j=j, m=m: _kvt_chain(g, j, m)))
                for st in range(ST):
                    a1.append((lambda st=st: _kvn_chain(g, st)))
                for i, ch in enumerate(a1):
                    ch()
                    if i % 2 == 1 and pending:
                        pending.pop(0)()

                wo_t = []
                for hh in range(G_H):
                    t = wp.tile([P, D], f32r, tag=f"wo_{hh}", name=f"wo_{g}_{hh}")
                    nc.sync.dma_start(t[:], wo[(g * G_H + hh) * P:
                                               (g * G_H + hh + 1) * P, :])
                    wo_t.append(t)

                # Phase B(g): attention pairs with interleaved o_proj drain
                for qp in range(N_QP):
                    qpsl = slice(qp * 2 * QT_W, (qp + 1) * 2 * QT_W)
                    for hh in range(G_H):
                        hg = g * G_H + hh
                        qs = smp.tile([P, 2 * QT_W], f32r, tag="qs", bufs=2,
                                      name=f"qs_{g}_{hh}_{qp}")
                        nc.sync.dma_start(qs[:], qTd[hg * P:(hg + 1) * P, qpsl])
                        ps_o = [psp.tile([P, QT_W], f32, tag="po", bufs=2,
                                         name=f"pso_{g}_{hh}_{qp}_{i}")
                                for i in range(2)]
                        acc_d = [wp.tile([P, QT_W], f32, tag=f"wq_{4 + i}",
                                         name=f"daccd_{g}_{hh}_{qp}_{i}")
                                 for i in range(2)]
                        acc_g = [wp.tile([P, QT_W], f32, tag=f"wq_{8 + i}",
                                         name=f"daccg_{g}_{hh}_{qp}_{i}")
                                 for i in range(2)]
                        es = [None] * KT

                        def _consume(kt):
                            e = es[kt]
                            nc.tensor.matmul(ps_o[0][:],
                                             kvn[kt][:, hh * P:(hh + 1) * P],
                                             e[:, :QT_W],
                                             start=(kt == 0), stop=(kt == KT - 1))
                            nc.tensor.matmul(ps_o[1][:],
                                             kvn[kt][:, hh * P:(hh + 1) * P],
                                             e[:, QT_W:],
                                             start=(kt == 0), stop=(kt == KT - 1))
                            acc = acc_d if kt % 2 == 0 else acc_g
                            for i in range(2):
                                esl = e[:, i * QT_W:(i + 1) * QT_W].bitcast(f32)
                                if kt < 2:
                                    nc.vector.tensor_copy(acc[i][:], esl)
                                else:
                                    nc.vector.tensor_add(acc[i][:], acc[i][:],
                                                         esl)

                        for kt in range(KT):
                            ps_s = psp.tile([P, 2 * QT_W], f32, tag="sc", bufs=2,
                                            name=f"pss_{g}_{hh}_{qp}_{kt}")
                            nc.tensor.matmul(ps_s[:, :QT_W],
                                             kvT[hh][:, kt * P:(kt + 1) * P],
                                             qs[:, :QT_W],
                                             start=True, stop=True)
                            nc.tensor.matmul(ps_s[:, QT_W:],
                                             kvT[hh][:, kt * P:(kt + 1) * P],
                                             qs[:, QT_W:],
                                             start=True, stop=True)
                            e = smp.tile([P, 2 * QT_W], f32r, tag="e", bufs=4,
                                         name=f"e_{g}_{hh}_{qp}_{kt}")
                            nc.scalar.activation(e[:], ps_s[:], EXP, scale=SCALE)
                            es[kt] = e
                            if kt >= 1:
                                _consume(kt - 1)
                            if kt % 2 == 1 and pending:
                                pending.pop(0)()
                        _consume(KT - 1)

                        for i in range(2):
                            acc_rd = wp.tile([P, QT_W], f32r, tag="wq_6",
                                             name=f"daccrd_{g}_{hh}_{qp}_{i}")
                            nc.vector.tensor_copy(acc_rd[:], acc_d[i][:])
                            acc_rg = wp.tile([P, QT_W], f32r, tag="wq_7",
                                             name=f"daccrg_{g}_{hh}_{qp}_{i}")
                            nc.vector.tensor_copy(acc_rg[:], acc_g[i][:])
                            ps_d = psp.tile([P, QT_W], f32, tag="pd", bufs=2,
                                            name=f"psd_{g}_{hh}_{qp}_{i}")
                            nc.tensor.matmul(ps_d[:], ones_t[:], acc_rd[:],
                                             start=True, stop=False)
                            nc.tensor.matmul(ps_d[:], ones_t[:], acc_rg[:],
                                             start=False, stop=True)
                            rcp = wp.tile([P, QT_W], f32, tag=f"wq_{14 + i}",
                                          name=f"rcp_{g}_{hh}_{qp}_{i}")
                            nc.vector.reciprocal_approx_fast(out=rcp[:],
                                                             in_=ps_d[:])
                            nc.vector.tensor_mul(
                                outT[hh][:, qp * 2 * QT_W + i * QT_W:
                                         qp * 2 * QT_W + (i + 1) * QT_W],
                                ps_o[i][:], rcp[:])

                    # queue o_proj chains for this (g, qp)
                    def _c_chain(g, st, np_i, ptag="pd", outT=outT, wo_t=wo_t):
                        pcs = [psp.tile([P, 2 * QT_W] if ptag == "sc"
                                        else [P, NT_W], f32, tag=ptag, bufs=2,
                                        name=f"psC_{g}_{st}_{np_i}_{i}")
                               for i in range(2)]
                        if ptag == "sc":
                            pcs = [p[:, :NT_W] for p in pcs]
                        for hh in range(G_H):
                            for i in range(2):
                                nt = 2 * np_i + i
                                nc.tensor.matmul(
                                    pcs[i][:],
                                    outT[hh][:, st * P:(st + 1) * P],
                                    wo_t[hh][:, nt * NT_W:(nt + 1) * NT_W],
                                    start=(hh == 0), stop=(hh == G_H - 1))
                        for i in range(2):
                            nt = 2 * np_i + i
                            fin = wp.tile([P, NT_W], f32,
                                          tag=f"wq_{10 + 2 * np_i + i}",
                                          name=f"fin_{g}_{st}_{nt}")
                            if nt % 2 == 0:
                                nc.vector.tensor_copy(fin[:], pcs[i][:])
                            else:
                                nc.scalar.copy(fin[:], pcs[i][:])
                            nc.gpsimd.dma_start(
                                outs[g][st * P:(st + 1) * P,
                                        nt * NT_W:(nt + 1) * NT_W],
                                fin[:])

                    for st in range(qp * 8, (qp + 1) * 8):
                        for np_i in range(N_NT // 2):
                            pending.append(
                                lambda ptag="pd", st=st, np_i=np_i, g=g:
                                _c_chain(g, st, np_i, ptag))

            # drain remaining o_proj chains (alternate psum tags for overlap)
            for idx, ch in enumerate(pending):
                ch("sc" if idx % 2 else "pd")
            pending = []

    nc.compile()
    return nc


_NC_CACHE = None


def _get_nc():
    global _NC_CACHE
    if _NC_CACHE is None:
        _NC_CACHE = _build_nc()
    return _NC_CACHE


def _run(x, W_q, W_kv_down, W_kv_up, W_o, trace=False):
    from concourse.bass_utils import run_bass_kernel_spmd

    nc = _get_nc()

    wdown_r = _round_fp32r(W_kv_down)
    wq_r = _round_fp32r(W_q)
    wup_r = _round_fp32r(W_kv_up)
    wo_r = _round_fp32r(W_o)
    ones = np.ones((P, P), np.float32)
    xT_b = [_round_fp32r(np.ascontiguousarray(x[b].T, dtype=np.float32))
            for b in range(B)]

    in_maps = []
    for c in range(N_CORES):
        bc = c // 4
        hs = slice((c % 4) * HW, (c % 4 + 1) * HW)
        in_maps.append({
            "xT": xT_b[bc],
            "wq": np.ascontiguousarray(wq_r[:, hs]),
            "wdown": wdown_r,
            "wup": np.ascontiguousarray(wup_r[:, hs]),
            "wo": np.ascontiguousarray(wo_r[hs, :]),
            "ones": ones,
        })

    r = run_bass_kernel_spmd(nc, in_maps, list(range(N_CORES)), trace=trace)
    outs = []
    for bc in range(B):
        acc = None
        for i in range(4):
            for g in range(N_G):
                part = r.results[4 * bc + i][f"out{g}"].astype(np.float64)
                acc = part if acc is None else acc + part
        outs.append(acc)
    return np.stack(outs).astype(np.float32), r


def kernel(x, W_q, W_kv_down, W_kv_up, W_o):
    out, _ = _run(x, W_q, W_kv_down, W_kv_up, W_o, trace=False)
    return out


# revision 14
# speedup vs baseline: 1.0343x; 1.0343x over previous
"""Multi-Head Latent Attention (MLA) TRN2 Bass kernel, 8-core parallel.

Sharding: batch x heads. Cores 0-3 own batch 0, cores 4-7 batch 1; within a
batch group each core owns 4 heads (tensor-parallel on q/kv_up/o_proj).
Each core computes the latent projection for its batch (4x replicated),
q/kv projections for its heads, attention, and a partial o_proj; the host
sums the 4 partials per batch and stacks the batches.

Dataflow is fully "transposed" so no on-device transposes are needed:
  xT [D, S] (host-side transpose, per batch) ->
  latentT = Wdown^T xT, qT = Wq^T xT, kvT = Wup^T latentT   (all [*, S])
  kv_nat = latentT^T Wup                                    ([S, *])
  scoresT[keys, q] = kvT^T(slice) qT;  expT = exp(scoresT * scale)
  outT[Dh, q]  = kv_nat^T(slice) expT  (accumulate over key tiles)
  denom[*, q]  = ones^T expT           (accumulate over key tiles)
  outT_norm    = outT * (1/denom);  final = outT_norm^T Wo   ([S, D])
Softmax max-subtraction is skipped: scores are ~N(0, 0.037), |s| < ~1.5.

The 4 heads run in 2 groups of 2 to fit SBUF; qT is staged through internal
DRAM; o_proj runs per group (group 0 writes, group 1 DMA-accumulates into
the pre-zeroed output) and is interleaved per q-chunk to spread the output
DMA and keep the PE dense (HAM-warm).

All matmuls run in float32r (fp32 with 11-bit mantissa, full PE speed).
"""
import sys

sys.path.insert(0, "/opt/trn_rl_repo")

import numpy as np  # noqa: E402

B = 2
S = 2048
D = 2048
H = 16
DH = 128
DL = 512
P = 128
N_CORES = 8
H_LOC = 4                     # heads per core
HW = H_LOC * DH               # 512
N_G = 2                       # head groups per core
G_H = H_LOC // N_G            # 2 heads per group
GW = G_H * DH                 # 256
SCALE = float(1.0 / np.sqrt(DH))

D_T = D // P                  # 16
L_T = DL // P                 # 4
S_SL = 256                    # projection s-slice width
N_SL = S // S_SL              # 8
QT_W = 512
N_QP = S // (2 * QT_W)        # 2 qt-pairs
KT = S // P                   # 16
ST = S // P                   # 16
NT_W = 512
N_NT = D // NT_W              # 4


def _round_fp32r(x: np.ndarray) -> np.ndarray:
    """Round fp32 to fp32r (11-bit mantissa, RNE), matching device rounding."""
    u = np.ascontiguousarray(x, dtype=np.float32).view(np.uint32)
    low = u & np.uint32(0x00000FFF)
    lsb = (u >> np.uint32(12)) & np.uint32(1)
    round_up = (low > np.uint32(0x800)) | ((low == np.uint32(0x800)) & (lsb == 1))
    out = (u & np.uint32(0xFFFFF000)) + (round_up.astype(np.uint32) << np.uint32(12))
    return out.view(np.float32)


def _build_nc():
    import concourse.tile as tile
    import concourse.mybir as mybir
    from concourse import bacc

    f32 = mybir.dt.float32
    f32r = mybir.dt.float32r
    EXP = mybir.ActivationFunctionType.Exp

    nc = bacc.Bacc("TRN2", target_bir_lowering=False, debug=False)

    xT = nc.dram_tensor("xT", [D, S], f32r, kind="ExternalInput").ap()
    wq = nc.dram_tensor("wq", [D, HW], f32r, kind="ExternalInput").ap()
    wdown = nc.dram_tensor("wdown", [D, DL], f32r, kind="ExternalInput").ap()
    wup = nc.dram_tensor("wup", [DL, HW], f32r, kind="ExternalInput").ap()
    wo = nc.dram_tensor("wo", [HW, D], f32r, kind="ExternalInput").ap()
    ones_d = nc.dram_tensor("ones", [P, P], f32r, kind="ExternalInput").ap()
    outs = [nc.dram_tensor(f"out{g}", [S, D], f32, kind="ExternalOutput").ap()
            for g in range(N_G)]

    with tile.TileContext(nc) as tc:
        with tc.tile_pool(name="w", bufs=1) as wp, \
             tc.tile_pool(name="xs", bufs=1) as xsp, \
             tc.tile_pool(name="big", bufs=1) as bigp, \
             tc.tile_pool(name="sm", bufs=1) as smp, \
             tc.tile_pool(name="dr", bufs=1, space="DRAM") as drp, \
             tc.tile_pool(name="ps", bufs=1, space="PSUM") as psp:

            qTd = drp.tile([HW, S], f32r, tag="qTd", name="qTd")

            # ---- weights + first-slice xs, in need order ----
            wdown_t = []
            xs0 = []
            for dt_i in range(D_T):
                t = wp.tile([P, DL], f32r, tag=f"wd_{dt_i}", name=f"wd_{dt_i}")
                nc.sync.dma_start(t[:], wdown[dt_i * P:(dt_i + 1) * P, :])
                wdown_t.append(t)
                t = xsp.tile([P, S_SL], f32r, tag=f"xs_{dt_i}", bufs=2,
                             name=f"xs_0_{dt_i}")
                nc.sync.dma_start(t[:], xT[dt_i * P:(dt_i + 1) * P, 0:S_SL])
                xs0.append(t)
            wq_t = []
            for dt_i in range(D_T):
                t = wp.tile([P, HW], f32r, tag=f"wq_{dt_i}", name=f"wq_{dt_i}")
                nc.sync.dma_start(t[:], wq[dt_i * P:(dt_i + 1) * P, :])
                wq_t.append(t)
            ones_t = wp.tile([P, P], f32r, tag="ones", name="ones")
            nc.sync.dma_start(ones_t[:], ones_d[:, :])

            latT = [bigp.tile([P, S], f32r, tag=f"latT_{m}", name=f"latT_{m}")
                    for m in range(L_T)]

            # ---- Phase A0: latent + q projections, streamed over s-slices
            for j in range(N_SL):
                if j == 0:
                    xs = xs0
                else:
                    xs = []
                    for dt_i in range(D_T):
                        t = xsp.tile([P, S_SL], f32r, tag=f"xs_{dt_i}", bufs=2,
                                     name=f"xs_{j}_{dt_i}")
                        nc.sync.dma_start(t[:], xT[dt_i * P:(dt_i + 1) * P,
                                                   j * S_SL:(j + 1) * S_SL])
                        xs.append(t)
                jsl = slice(j * S_SL, (j + 1) * S_SL)
                for m in range(L_T):
                    ps = psp.tile([P, 2 * QT_W], f32, tag="sc", bufs=2,
                                  name=f"psA_{j}_l{m}")
                    for dt_i in range(D_T):
                        nc.tensor.matmul(ps[:, :S_SL],
                                         wdown_t[dt_i][:, m * P:(m + 1) * P],
                                         xs[dt_i][:],
                                         start=(dt_i == 0),
                                         stop=(dt_i == D_T - 1))
                    nc.vector.tensor_copy(latT[m][:, jsl], ps[:, :S_SL])
                for m in range(H_LOC):
                    ps = psp.tile([P, 2 * QT_W], f32, tag="sc", bufs=2,
                                  name=f"psA_{j}_q{m}")
                    for dt_i in range(D_T):
                        nc.tensor.matmul(ps[:, :S_SL],
                                         wq_t[dt_i][:, m * P:(m + 1) * P],
                                         xs[dt_i][:],
                                         start=(dt_i == 0),
                                         stop=(dt_i == D_T - 1))
                    qs_t = smp.tile([P, S_SL], f32r, tag="qstage", bufs=3,
                                    name=f"qstage_{j}_{m}")
                    nc.vector.tensor_copy(qs_t[:], ps[:, :S_SL])
                    nc.gpsimd.dma_start(qTd[m * P:(m + 1) * P, jsl], qs_t[:])

            # ---- per head-group: kv projections, attention, o_proj ----
            # o_proj chains are queued and drained one-per-2-kt inside later
            # attention loops (and the next group's kv loop) so their PE/DVE/
            # ACT/DMA work fills dependency-stall gaps instead of serializing.
            pending = []

            for g in range(N_G):
                gsl = slice(g * GW, (g + 1) * GW)
                wup_t = []
                for lt in range(L_T):
                    t = wp.tile([P, GW], f32r, tag=f"wu_{lt}", name=f"wu_{g}_{lt}")
                    nc.sync.dma_start(t[:], wup[lt * P:(lt + 1) * P, gsl])
                    wup_t.append(t)

                kvT = [bigp.tile([P, S], f32r, tag=f"kvT_{m}", name=f"kvT_{g}_{m}")
                       for m in range(G_H)]
                # kv_nat shares the wd_* weight slots (free after Phase A0)
                kvn = [wp.tile([P, GW], f32r, tag=f"wd_{st}", name=f"kvn_{g}_{st}")
                       for st in range(ST)]
                outT = [bigp.tile([P, S], f32r, tag=f"outT_{m}",
                                  name=f"outT_{g}_{m}")
                        for m in range(G_H)]

                # Phase A1(g): kv projections from latT, drain pending o_proj
                def _kvt_chain(g, j, m, kvT=kvT, wup_t=wup_t):
                    jsl = slice(j * S_SL, (j + 1) * S_SL)
                    ps = psp.tile([P, 2 * QT_W], f32, tag="sc", bufs=2,
                                  name=f"psK_{g}_{j}_{m}")
                    for lt in range(L_T):
                        nc.tensor.matmul(ps[:, :S_SL],
                                         wup_t[lt][:, m * P:(m + 1) * P],
                                         latT[lt][:, jsl],
                                         start=(lt == 0), stop=(lt == L_T - 1))
                    nc.vector.tensor_copy(kvT[m][:, jsl], ps[:, :S_SL])

                def _kvn_chain(g, st, kvn=kvn, wup_t=wup_t):
                    ps = psp.tile([P, 2 * QT_W], f32, tag="sc", bufs=2,
                                  name=f"psN_{g}_{st}")
                    for lt in range(L_T):
                        nc.tensor.matmul(ps[:, :GW],
                                         latT[lt][:, st * P:(st + 1) * P],
                                         wup_t[lt][:],
                                         start=(lt == 0), stop=(lt == L_T - 1))
                    nc.vector.tensor_copy(kvn[st][:], ps[:, :GW])

                a1 = []
                for j in range(N_SL):
                    for m in range(G_H):
                        a1.append((lambda j=j, m=m: _kvt_chain(g, j, m)))
                for st in range(ST):
                    a1.append((lambda st=st: _kvn_chain(g, st)))
                for i, ch in enumerate(a1):
                    ch()
                    if i % 2 == 1 and pending:
                        pending.pop(0)()

                wo_t = []
                for hh in range(G_H):
                    t = wp.tile([P, D], f32r, tag=f"wo_{hh}", name=f"wo_{g}_{hh}")
                    nc.sync.dma_start(t[:], wo[(g * G_H + hh) * P:
                                               (g * G_H + hh + 1) * P, :])
                    wo_t.append(t)

                # Phase B(g): attention pairs with interleaved o_proj drain
                for qp in range(N_QP):
                    qpsl = slice(qp * 2 * QT_W, (qp + 1) * 2 * QT_W)
                    for hh in range(G_H):
                        hg = g * G_H + hh
                        qs = smp.tile([P, 2 * QT_W], f32r, tag="qs", bufs=2,
                                      name=f"qs_{g}_{hh}_{qp}")
                        nc.sync.dma_start(qs[:], qTd[hg * P:(hg + 1) * P, qpsl])
                        ps_o = [psp.tile([P, QT_W], f32, tag="po", bufs=2,
                                         name=f"pso_{g}_{hh}_{qp}_{i}")
                                for i in range(2)]
                        acc_d = [wp.tile([P, QT_W], f32, tag=f"wq_{4 + i}",
                                         name=f"daccd_{g}_{hh}_{qp}_{i}")
                                 for i in range(2)]
                        acc_g = [wp.tile([P, QT_W], f32, tag=f"wq_{8 + i}",
                                         name=f"daccg_{g}_{hh}_{qp}_{i}")
                                 for i in range(2)]
                        es = [None] * KT

                        def _consume(kt):
                            e = es[kt]
                            nc.tensor.matmul(ps_o[0][:],
                                             kvn[kt][:, hh * P:(hh + 1) * P],
                                             e[:, :QT_W],
                                             start=(kt == 0), stop=(kt == KT - 1))
                            nc.tensor.matmul(ps_o[1][:],
                                             kvn[kt][:, hh * P:(hh + 1) * P],
                                             e[:, QT_W:],
                                             start=(kt == 0), stop=(kt == KT - 1))
                            acc = acc_d if kt % 2 == 0 else acc_g
                            for i in range(2):
                                esl = e[:, i * QT_W:(i + 1) * QT_W].bitcast(f32)
                                if kt < 2:
                                    nc.vector.tensor_copy(acc[i][:], esl)
                                else:
                                    nc.vector.tensor_add(acc[i][:], acc[i][:],
                                                         esl)

                        for kt in range(KT):
                            ps_s = psp.tile([P, 2 * QT_W], f32, tag="sc", bufs=2,
                                            name=f"pss_{g}_{hh}_{qp}_{kt}")
                            nc.tensor.matmul(ps_s[:, :QT_W],
                                             kvT[hh][:, kt * P:(kt + 1) * P],
                                             qs[:, :QT_W],
                                             start=True, stop=True)
                            nc.tensor.matmul(ps_s[:, QT_W:],
                                             kvT[hh][:, kt * P:(kt + 1) * P],
                                             qs[:, QT_W:],
                                             start=True, stop=True)
                            e = smp.tile([P, 2 * QT_W], f32r, tag="e", bufs=3,
                                         name=f"e_{g}_{hh}_{qp}_{kt}")
                            nc.scalar.activation(e[:], ps_s[:], EXP, scale=SCALE)
                            es[kt] = e
                            if kt >= 1:
                                _consume(kt - 1)
                            if kt % 2 == 1 and pending:
                                pending.pop(0)()
                        _consume(KT - 1)

                        for i in range(2):
                            acc_rd = wp.tile([P, QT_W], f32r, tag="wq_6",
                                             name=f"daccrd_{g}_{hh}_{qp}_{i}")
                            nc.vector.tensor_copy(acc_rd[:], acc_d[i][:])
                            acc_rg = wp.tile([P, QT_W], f32r, tag="wq_7",
                                             name=f"daccrg_{g}_{hh}_{qp}_{i}")
                            nc.vector.tensor_copy(acc_rg[:], acc_g[i][:])
                            ps_d = psp.tile([P, QT_W], f32, tag="pd", bufs=2,
                                            name=f"psd_{g}_{hh}_{qp}_{i}")
                            nc.tensor.matmul(ps_d[:], ones_t[:], acc_rd[:],
                                             start=True, stop=False)
                            nc.tensor.matmul(ps_d[:], ones_t[:], acc_rg[:],
                                             start=False, stop=True)
                            rcp = wp.tile([P, QT_W], f32, tag=f"wq_{14 + i}",
                                          name=f"rcp_{g}_{hh}_{qp}_{i}")
                            nc.vector.reciprocal_approx_fast(out=rcp[:],
                                                             in_=ps_d[:])
                            nc.vector.tensor_mul(
                                outT[hh][:, qp * 2 * QT_W + i * QT_W:
                                         qp * 2 * QT_W + (i + 1) * QT_W],
                                ps_o[i][:], rcp[:])

                    # queue o_proj chains for this (g, qp)
                    def _c_chain(g, st, np_i, ptag="pd", outT=outT, wo_t=wo_t):
                        pcs = [psp.tile([P, 2 * QT_W] if ptag == "sc"
                                        else [P, NT_W], f32, tag=ptag, bufs=2,
                                        name=f"psC_{g}_{st}_{np_i}_{i}")
                               for i in range(2)]
                        if ptag == "sc":
                            pcs = [p[:, :NT_W] for p in pcs]
                        for hh in range(G_H):
                            for i in range(2):
                                nt = 2 * np_i + i
                                nc.tensor.matmul(
                                    pcs[i][:],
                                    outT[hh][:, st * P:(st + 1) * P],
                                    wo_t[hh][:, nt * NT_W:(nt + 1) * NT_W],
                                    start=(hh == 0), stop=(hh == G_H - 1))
                        for i in range(2):
                            nt = 2 * np_i + i
                            fin = wp.tile([P, NT_W], f32,
                                          tag=f"wq_{10 + 2 * np_i + i}",
                                          name=f"fin_{g}_{st}_{nt}")
                            if nt % 2 == 0:
                                nc.vector.tensor_copy(fin[:], pcs[i][:])
                            else:
                                nc.scalar.copy(fin[:], pcs[i][:])
                            nc.gpsimd.dma_start(
                                outs[g][st * P:(st + 1) * P,
                                        nt * NT_W:(nt + 1) * NT_W],
                                fin[:])

                    for st in range(qp * 8, (qp + 1) * 8):
                        for np_i in range(N_NT // 2):
                            pending.append(
                                lambda ptag="pd", st=st, np_i=np_i, g=g:
                                _c_chain(g, st, np_i, ptag))

            # drain remaining o_proj chains (alternate psum tags for overlap)
            for idx, ch in enumerate(pending):
                ch("sc" if idx % 2 else "pd")
            pending = []

    nc.compile()
    return nc


_NC_CACHE = None


def _get_nc():
    global _NC_CACHE
    if _NC_CACHE is None:
        _NC_CACHE = _build_nc()
    return _NC_CACHE


def _run(x, W_q, W_kv_down, W_kv_up, W_o, trace=False):
    from concourse.bass_utils import run_bass_kernel_spmd

    nc = _get_nc()

    wdown_r = _round_fp32r(W_kv_down)
    wq_r = _round_fp32r(W_q)
    wup_r = _round_fp32r(W_kv_up)
    wo_r = _round_fp32r(W_o)
    ones = np.ones((P, P), np.float32)
    xT_b = [_round_fp32r(np.ascontiguousarray(x[b].T, dtype=np.float32))
            for b in range(B)]

    in_maps = []
    for c in range(N_CORES):
        bc = c // 4
        hs = slice((c % 4) * HW, (c % 4 + 1) * HW)
        in_maps.append({
            "xT": xT_b[bc],
            "wq": np.ascontiguousarray(wq_r[:, hs]),
            "wdown": wdown_r,
            "wup": np.ascontiguousarray(wup_r[:, hs]),
            "wo": np.ascontiguousarray(wo_r[hs, :]),
            "ones": ones,
        })

    r = run_bass_kernel_spmd(nc, in_maps, list(range(N_CORES)), trace=trace)
    outs = []
    for bc in range(B):
        acc = None
        for i in range(4):
            for g in range(N_G):
                part = r.results[4 * bc + i][f"out{g}"].astype(np.float64)
                acc = part if acc is None else acc + part
        outs.append(acc)
    return np.stack(outs).astype(np.float32), r


def kernel(x, W_q, W_kv_down, W_kv_up, W_o):
    out, _ = _run(x, W_q, W_kv_down, W_kv_up, W_o, trace=False)
    return out
